# revision 17
# baseline (speedup 1.0000x reference)
"""Trainium2 Bass kernel for nn_Attention (general-score attention with
masked softmax), data-parallel over batch across 8 NeuronCores.

Math (per batch), matching the reference exactly for {0,1} float masks:
    raw[t,s]  = sum_e (hidden @ W)[t,e] * enc[s,e]       (associativity trick:
                (hidden @ W) @ enc^T  ==  hidden @ (enc @ W^T)^T, saves 25%
                FLOPs and avoids materializing proj)
    attn_energies = raw * mask            (mask in {0,1} so mask^2 == mask)
    e = exp(x - max_s x) * mask
    attn = e / (sum_s e + 1e-6)
    context = attn @ enc_value

Layouts: host marshals hidden^T (D,T) and enc^T (E,S) per batch so every
matmul contracts over the partition dim with zero on-device transposes,
except attn^T which is produced on-device via PE transpose (bf16).
mm1/mm2 run in float32r (e8m11; 1 cycle/row at N>=512 vs 4 for plain f32)
to keep the softmax exponents accurate; the attention tail (attn, val, mm3)
runs in bf16. Outputs ae/ctx/aw are rounded to bf16 on device and widened
to f32 on the host. Measured end-to-end rel err ~2.4e-3.

Schedule (two batches per core, software-pipelined):
  b0: loads -> mm1 (dt-outer over 8 psum banks, consumes DMA slices as they
      land) -> mm2 for all 4 t-tiles (groups kept sequential with explicit
      deps; softmax chains pipeline behind on DVE/ACT) -> per t-tile PE
      transposes + mm3, with b1's mm1 (et-outer, psB halves) interleaved
      between t-tiles so the PE never waits on a softmax chain.
  b1: same minus the interleaved successor.
"""
import os

import ml_dtypes
import numpy as np

B, TRG, SRC, ENCD, TRGD = 16, 512, 1024, 1024, 1024
NCORES = 8
BPC = B // NCORES  # batches per core
P = 128
nD = TRGD // P   # 8 contraction tiles over d
nE = ENCD // P   # 8 over e
nS = SRC // P    # 8 over s
nT = TRG // P    # 4 t-tiles

_cache = {}

LAST_EXEC_NS = None
LAST_RESULTS = None


def _build():
    import bass_rust
    import concourse.mybir as mybir
    import concourse.tile as tile
    from concourse import bacc
    from concourse.masks import make_identity

    _add_dep = bass_rust.add_dep_helper

    F32 = mybir.dt.float32
    F32R = mybir.dt.float32r
    BF16 = mybir.dt.bfloat16
    ALU = mybir.AluOpType
    AXL = mybir.AxisListType
    ACT_EXP = mybir.ActivationFunctionType.Exp

    nc = bacc.Bacc("TRN2", target_bir_lowering=False, debug=False)

    hidT_d = nc.dram_tensor("hidT", (BPC, TRGD, TRG), F32R, kind="ExternalInput")
    w_d = nc.dram_tensor("w", (TRGD, ENCD), F32R, kind="ExternalInput")
    encT_d = nc.dram_tensor("encT", (BPC, ENCD, SRC), F32R, kind="ExternalInput")
    val_d = nc.dram_tensor("val", (BPC, SRC, TRGD), BF16, kind="ExternalInput")
    mask_d = nc.dram_tensor("mask", (BPC, 1, SRC), F32, kind="ExternalInput")
    ae_d = nc.dram_tensor("ae", (BPC, TRG, SRC), BF16, kind="ExternalOutput")
    aw_d = nc.dram_tensor("aw", (BPC, TRG, SRC), BF16, kind="ExternalOutput")
    ctx_d = nc.dram_tensor("ctx", (BPC, TRG, TRGD), BF16, kind="ExternalOutput")

    with tile.TileContext(nc) as tc:
        with (
            tc.tile_pool(name="const", bufs=1) as const,
            tc.tile_pool(name="wp", bufs=1) as wp,
            tc.tile_pool(name="big", bufs=1) as big,
            tc.tile_pool(name="sm", bufs=2) as sm,
            tc.tile_pool(name="xs", bufs=4) as xs,
            tc.tile_pool(name="psA", bufs=2, space="PSUM") as psA,
            tc.tile_pool(name="psB", bufs=3, space="PSUM") as psB,
        ):
            ident = const.tile([P, P], F32)
            make_identity(nc, ident[:])
            identb = const.tile([P, P], BF16)
            nc.vector.tensor_copy(identb[:], ident[:])

            w_sb = [wp.tile([P, ENCD], F32R, tag=f"w{i}", name=f"w_sb{i}")
                    for i in range(nD)]

            def emit_loads(b):
                hidT_sb = [big.tile([P, TRG], F32R, tag=f"hidT{i}",
                                    name=f"hidT_sb{i}") for i in range(nD)]
                # DMA issue order == consumption order for the b0 ramp
                for i in range(nD):
                    if b == 0:
                        nc.sync.dma_start(out=w_sb[i][:],
                                          in_=w_d[i * P:(i + 1) * P, :])
                    nc.sync.dma_start(out=hidT_sb[i][:],
                                      in_=hidT_d[b, i * P:(i + 1) * P, :])
                maskb = sm.tile([P, SRC], F32, tag="maskb")
                nc.sync.dma_start(out=maskb[:],
                                  in_=mask_d[b].to_broadcast((P, SRC)))
                encT_sb = big.tile([P, nE, SRC], F32R, tag="encT")
                for i in range(nE):
                    nc.sync.dma_start(out=encT_sb[:, i, :],
                                      in_=encT_d[b, i * P:(i + 1) * P, :])
                val_sb = big.tile([P, nS, TRGD], BF16, tag="val")
                for i in range(nS):
                    nc.sync.dma_start(out=val_sb[:, i, :],
                                      in_=val_d[b, i * P:(i + 1) * P, :])
                return hidT_sb, maskb, encT_sb, val_sb

            def emit_mm1_ramp(hidT_sb):
                """b0: dt-outer over 8 concurrent psum groups; each
                (w[dt], hidT[dt]) pair is consumed as its DMA lands; the
                et-outer second half staggers the HpT copies on DVE."""
                HpT = big.tile([P, nE, TRG], F32R, tag="HpT", name="HpT0")
                mm1_ps = [psB.tile([P, SRC], F32, tag="ps_b",
                                   name=f"mm1ps{j}") for j in range(3)]
                mm1_ps2 = [psA.tile([P, TRG], F32, tag="ps_a",
                                    name=f"mm1ps2{j}") for j in range(2)]

                def et_psum(et):
                    if et < 6:
                        return mm1_ps[et // 2][:, (et % 2) * 512:
                                               (et % 2 + 1) * 512]
                    return mm1_ps2[et - 6][:]

                for dt in range(nD // 2):
                    for et in range(nE):
                        nc.tensor.matmul(et_psum(et),
                                         w_sb[dt][:, et * P:(et + 1) * P],
                                         hidT_sb[dt][:],
                                         start=(dt == 0), stop=False)
                for et in range(nE):
                    for dt in range(nD // 2, nD):
                        nc.tensor.matmul(et_psum(et),
                                         w_sb[dt][:, et * P:(et + 1) * P],
                                         hidT_sb[dt][:],
                                         start=False, stop=(dt == nD - 1))
                    nc.vector.tensor_copy(HpT[:, et, :], et_psum(et))
                return HpT

            def mm1_chunks(hidT_sb):
                """b>0: et-outer groups (2 per chunk) on psB halves, meant to
                be interleaved into the previous batch's transpose/mm3 phase
                (which only holds one psB slot at a time)."""
                HpT = big.tile([P, nE, TRG], F32R, tag="HpT", name="HpT1")

                def chunk(et_pair):
                    def emit():
                        for et in et_pair:
                            pp = psB.tile([P, SRC], F32, tag="ps_b",
                                          name=f"mm1b_ps{et}")
                            half = pp[:, :TRG]
                            for dt in range(nD):
                                nc.tensor.matmul(
                                    half, w_sb[dt][:, et * P:(et + 1) * P],
                                    hidT_sb[dt][:],
                                    start=(dt == 0), stop=(dt == nD - 1))
                            nc.vector.tensor_copy(HpT[:, et, :], half)
                    return emit
                return HpT, [chunk((2 * j, 2 * j + 1)) for j in range(nE // 2)]

            def emit_mm2_chains(b, HpT, maskb, encT_sb):
                attns = []
                prev_last_mm = None
                for tt in range(nT):
                    ts = slice(tt * P, (tt + 1) * P)
                    en_ps = psB.tile([P, SRC], F32, tag="ps_b")
                    first_mm = None
                    for et in range(nE):
                        for h in range(2):
                            hs = slice(h * 512, (h + 1) * 512)
                            mm = nc.tensor.matmul(en_ps[:, hs], HpT[:, et, ts],
                                                  encT_sb[:, et, hs],
                                                  start=(et == 0),
                                                  stop=(et == nE - 1))
                            if first_mm is None:
                                first_mm = mm
                            last_mm = mm
                    # keep mm2 groups sequential on PE: otherwise the
                    # scheduler interleaves groups and delays group 0's stop
                    # (and with it every softmax chain) by ~8us.
                    if prev_last_mm is not None:
                        _add_dep(first_mm.ins, prev_last_mm.ins, sync=False,
                                 reason="mm2 group order")
                    prev_last_mm = last_mm

                    x = xs.tile([P, SRC], F32, tag="x")
                    nc.vector.tensor_mul(x[:], en_ps[:], maskb[:])
                    ae_bf = sm.tile([P, SRC], BF16, tag="ae_bf")
                    nc.gpsimd.tensor_copy(ae_bf[:], x[:])
                    nc.sync.dma_start(out=ae_d[b, ts, :], in_=ae_bf[:])
                    negm = sm.tile([P, 1], F32, tag="negm")
                    nc.vector.tensor_reduce(negm[:], x[:], axis=AXL.X,
                                            op=ALU.max, negate=True)
                    ex = sm.tile([P, SRC], F32, tag="ex")
                    nc.scalar.activation(ex[:], x[:], ACT_EXP, bias=negm[:],
                                         scale=1.0)
                    rowsum = sm.tile([P, 1], F32, tag="rowsum")
                    nc.vector.scalar_tensor_tensor(ex[:], ex[:], 1.0, maskb[:],
                                                   op0=ALU.mult, op1=ALU.mult,
                                                   accum_out=rowsum[:])
                    z = sm.tile([P, 1], F32, tag="z")
                    nc.vector.tensor_scalar_add(z[:], rowsum[:], 1e-6)
                    rz = sm.tile([P, 1], F32, tag="rz")
                    nc.vector.reciprocal(rz[:], z[:])
                    attn = xs.tile([P, SRC], BF16, tag="attn")
                    nc.vector.tensor_scalar_mul(attn[:], ex[:], rz[:])
                    nc.sync.dma_start(out=aw_d[b, ts, :], in_=attn[:])
                    attns.append(attn)
                return attns

            def emit_tail(b, attns, val_sb, filler_chunks):
                """Per t-tile: PE transposes of attn + mm3; interleave the
                next batch's mm1 chunks between t-tiles."""
                for tt in range(nT):
                    ts = slice(tt * P, (tt + 1) * P)
                    attn = attns[tt]
                    attnT = sm.tile([P, nS, P], BF16, tag="attnT")
                    for st in range(nS):
                        pt = psA.tile([P, TRG], F32, tag="ps_a")
                        ptb = pt[:].bitcast(BF16)
                        nc.tensor.transpose(ptb[:, :P],
                                            attn[:, st * P:(st + 1) * P],
                                            identb[:])
                        if st % 2 == 0:
                            nc.vector.tensor_copy(attnT[:, st, :], ptb[:, :P])
                        else:
                            nc.scalar.copy(attnT[:, st, :], ptb[:, :P])

                    ctx_ps = psB.tile([P, TRGD], F32, tag="ps_b")
                    for st in range(nS):
                        for h in range(2):
                            hs = slice(h * 512, (h + 1) * 512)
                            nc.tensor.matmul(ctx_ps[:, hs], attnT[:, st, :],
                                             val_sb[:, st, hs],
                                             start=(st == 0),
                                             stop=(st == nS - 1))
                    ctx_sb = sm.tile([P, TRGD], BF16, tag="ctx_sb")
                    nc.scalar.copy(ctx_sb[:], ctx_ps[:])
                    nc.sync.dma_start(out=ctx_d[b, ts, :], in_=ctx_sb[:])

                    if filler_chunks:
                        filler_chunks.pop(0)()
                for ch in filler_chunks:
                    ch()

            # ---- two-batch pipeline ----
            hidT0, maskb0, encT0, val0 = emit_loads(0)
            HpT0 = emit_mm1_ramp(hidT0)
            attns0 = emit_mm2_chains(0, HpT0, maskb0, encT0)

            hidT1, maskb1, encT1, val1 = emit_loads(1)
            HpT1, chunks1 = mm1_chunks(hidT1)
            emit_tail(0, attns0, val0, chunks1)

            attns1 = emit_mm2_chains(1, HpT1, maskb1, encT1)
            emit_tail(1, attns1, val1, [])

    nc.compile()
    return nc


def kernel(hidden, encoder_outputs, encoder_value, encoder_mask, W):
    global LAST_EXEC_NS, LAST_RESULTS
    from concourse.bass_utils import run_bass_kernel_spmd

    if "nc" not in _cache:
        _cache["nc"] = _build()
    nc = _cache["nc"]

    hidden = np.ascontiguousarray(hidden, dtype=np.float32)
    encoder_outputs = np.ascontiguousarray(encoder_outputs, dtype=np.float32)
    encoder_value = np.ascontiguousarray(encoder_value, dtype=np.float32)
    encoder_mask = np.ascontiguousarray(encoder_mask, dtype=np.float32)
    W = np.ascontiguousarray(W, dtype=np.float32)

    in_maps = []
    for c in range(NCORES):
        sl = slice(c * BPC, (c + 1) * BPC)
        in_maps.append({
            "hidT": np.ascontiguousarray(hidden[sl].transpose(0, 2, 1)),
            "w": W,
            "encT": np.ascontiguousarray(encoder_outputs[sl].transpose(0, 2, 1)),
            "val": encoder_value[sl].astype(ml_dtypes.bfloat16),
            "mask": encoder_mask[sl][:, None, :],
        })

    trace = bool(int(os.environ.get("KERNEL_TRACE", "0")))
    res = run_bass_kernel_spmd(nc, in_maps, core_ids=list(range(NCORES)),
                               trace=trace)
    LAST_EXEC_NS = res.exec_time_ns
    LAST_RESULTS = res

    context = np.concatenate([res.results[c]["ctx"] for c in range(NCORES)],
                             axis=0).astype(np.float32)
    attn_weights = np.concatenate([res.results[c]["aw"] for c in range(NCORES)],
                                  axis=0).astype(np.float32)
    attn_energies = np.concatenate([res.results[c]["ae"] for c in range(NCORES)],
                                   axis=0).astype(np.float32)
    return context, attn_weights, attn_energies


# revision 18
# speedup vs baseline: 1.0957x; 1.0957x over previous
"""Trainium2 Bass kernel for nn_Attention (general-score attention with
masked softmax), data-parallel over batch across 8 NeuronCores.

Math (per batch), matching the reference exactly for {0,1} float masks:
    raw[t,s]  = sum_e (hidden @ W)[t,e] * enc[s,e]       (associativity trick:
                (hidden @ W) @ enc^T  ==  hidden @ (enc @ W^T)^T, saves 25%
                FLOPs and avoids materializing proj)
    attn_energies = raw * mask            (mask in {0,1} so mask^2 == mask)
    e = exp(x - max_s x) * mask
    attn = e / (sum_s e + 1e-6)
    context = attn @ enc_value

Layouts: host marshals hidden^T (D,T) and enc^T (E,S) per batch so every
matmul contracts over the partition dim with zero on-device transposes,
except attn^T which is produced on-device via PE transpose (bf16).
mm1/mm2 run in float32r (e8m11; 1 cycle/row at N>=512 vs 4 for plain f32)
to keep the softmax exponents accurate; the attention tail (attn, val, mm3)
runs in bf16. Outputs ae/ctx/aw are rounded to bf16 on device and widened
to f32 on the host. Measured end-to-end rel err ~2.4e-3.

Schedule (two batches per core, software-pipelined):
  b0: loads -> mm1 (dt-outer over 8 psum banks, consumes DMA slices as they
      land) -> mm2 for all 4 t-tiles (groups kept sequential with explicit
      deps; softmax chains pipeline behind on DVE/ACT) -> per t-tile PE
      transposes + mm3, with b1's mm1 (et-outer, psB halves) interleaved
      between t-tiles so the PE never waits on a softmax chain.
  b1: same minus the interleaved successor.
"""
import os

import ml_dtypes
import numpy as np

B, TRG, SRC, ENCD, TRGD = 16, 512, 1024, 1024, 1024
NCORES = 8
BPC = B // NCORES  # batches per core
P = 128
nD = TRGD // P   # 8 contraction tiles over d
nE = ENCD // P   # 8 over e
nS = SRC // P    # 8 over s
nT = TRG // P    # 4 t-tiles

_cache = {}

LAST_EXEC_NS = None
LAST_RESULTS = None


def _build():
    import bass_rust
    import concourse.mybir as mybir
    import concourse.tile as tile
    from concourse import bacc
    from concourse.masks import make_identity

    _add_dep = bass_rust.add_dep_helper

    F32 = mybir.dt.float32
    F32R = mybir.dt.float32r
    BF16 = mybir.dt.bfloat16
    ALU = mybir.AluOpType
    AXL = mybir.AxisListType
    ACT_EXP = mybir.ActivationFunctionType.Exp

    nc = bacc.Bacc("TRN2", target_bir_lowering=False, debug=False)

    hidT_d = nc.dram_tensor("hidT", (BPC, TRGD, TRG), F32R, kind="ExternalInput")
    w_d = nc.dram_tensor("w", (TRGD, ENCD), F32R, kind="ExternalInput")
    encT_d = nc.dram_tensor("encT", (BPC, ENCD, SRC), F32R, kind="ExternalInput")
    val_d = nc.dram_tensor("val", (BPC, SRC, TRGD), BF16, kind="ExternalInput")
    mask_d = nc.dram_tensor("mask", (BPC, 1, SRC), F32, kind="ExternalInput")
    ae_d = nc.dram_tensor("ae", (BPC, TRG, SRC), BF16, kind="ExternalOutput")
    aw_d = nc.dram_tensor("aw", (BPC, TRG, SRC), BF16, kind="ExternalOutput")
    ctx_d = nc.dram_tensor("ctx", (BPC, TRG, TRGD), BF16, kind="ExternalOutput")

    with tile.TileContext(nc) as tc:
        with (
            tc.tile_pool(name="const", bufs=1) as const,
            tc.tile_pool(name="wp", bufs=1) as wp,
            tc.tile_pool(name="big", bufs=1) as big,
            tc.tile_pool(name="sm", bufs=2) as sm,
            tc.tile_pool(name="xs", bufs=4) as xs,
            tc.tile_pool(name="psA", bufs=2, space="PSUM") as psA,
            tc.tile_pool(name="psB", bufs=3, space="PSUM") as psB,
        ):
            ident = const.tile([P, P], F32)
            make_identity(nc, ident[:])
            identb = const.tile([P, P], BF16)
            nc.vector.tensor_copy(identb[:], ident[:])

            w_sb = [wp.tile([P, ENCD], F32R, tag=f"w{i}", name=f"w_sb{i}")
                    for i in range(nD)]

            def emit_loads(b):
                hidT_sb = [big.tile([P, TRG], F32R, tag=f"hidT{i}",
                                    name=f"hidT_sb{i}") for i in range(nD)]
                # DMA issue order == consumption order for the b0 ramp
                for i in range(nD):
                    if b == 0:
                        nc.sync.dma_start(out=w_sb[i][:],
                                          in_=w_d[i * P:(i + 1) * P, :])
                    nc.sync.dma_start(out=hidT_sb[i][:],
                                      in_=hidT_d[b, i * P:(i + 1) * P, :])
                maskb = sm.tile([P, SRC], F32, tag="maskb")
                nc.sync.dma_start(out=maskb[:],
                                  in_=mask_d[b].to_broadcast((P, SRC)))
                encT_sb = big.tile([P, nE, SRC], F32R, tag="encT")
                for i in range(nE):
                    nc.sync.dma_start(out=encT_sb[:, i, :],
                                      in_=encT_d[b, i * P:(i + 1) * P, :])
                val_sb = big.tile([P, nS, TRGD], BF16, tag="val")
                for i in range(nS):
                    nc.sync.dma_start(out=val_sb[:, i, :],
                                      in_=val_d[b, i * P:(i + 1) * P, :])
                return hidT_sb, maskb, encT_sb, val_sb

            def emit_mm1_ramp(hidT_sb):
                """b0: dt-outer over 8 concurrent psum groups; each
                (w[dt], hidT[dt]) pair is consumed as its DMA lands; the
                et-outer second half staggers the HpT copies on DVE."""
                HpT = big.tile([P, nE, TRG], F32R, tag="HpT", name="HpT0")
                mm1_ps = [psB.tile([P, SRC], F32, tag="ps_b",
                                   name=f"mm1ps{j}") for j in range(3)]
                mm1_ps2 = [psA.tile([P, TRG], F32, tag="ps_a",
                                    name=f"mm1ps2{j}") for j in range(2)]

                def et_psum(et):
                    if et < 6:
                        return mm1_ps[et // 2][:, (et % 2) * 512:
                                               (et % 2 + 1) * 512]
                    return mm1_ps2[et - 6][:]

                for dt in range(nD // 2):
                    for et in range(nE):
                        nc.tensor.matmul(et_psum(et),
                                         w_sb[dt][:, et * P:(et + 1) * P],
                                         hidT_sb[dt][:],
                                         start=(dt == 0), stop=False)
                for et in range(nE):
                    for dt in range(nD // 2, nD):
                        nc.tensor.matmul(et_psum(et),
                                         w_sb[dt][:, et * P:(et + 1) * P],
                                         hidT_sb[dt][:],
                                         start=False, stop=(dt == nD - 1))
                    nc.vector.tensor_copy(HpT[:, et, :], et_psum(et))
                return HpT

            def mm1_chunks(hidT_sb):
                """b>0: et-outer groups (2 per chunk) on psB halves, meant to
                be interleaved into the previous batch's transpose/mm3 phase
                (which only holds one psB slot at a time)."""
                HpT = big.tile([P, nE, TRG], F32R, tag="HpT", name="HpT1")

                def chunk(et_pair):
                    def emit():
                        for et in et_pair:
                            pp = psB.tile([P, SRC], F32, tag="ps_b",
                                          name=f"mm1b_ps{et}")
                            half = pp[:, :TRG]
                            for dt in range(nD):
                                nc.tensor.matmul(
                                    half, w_sb[dt][:, et * P:(et + 1) * P],
                                    hidT_sb[dt][:],
                                    start=(dt == 0), stop=(dt == nD - 1))
                            nc.vector.tensor_copy(HpT[:, et, :], half)
                    return emit
                return HpT, [chunk((2 * j, 2 * j + 1)) for j in range(nE // 2)]

            def emit_mm2_chains(b, HpT, maskb, encT_sb):
                attns = []
                prev_last_mm = None
                for tt in range(nT):
                    ts = slice(tt * P, (tt + 1) * P)
                    en_ps = psB.tile([P, SRC], F32, tag="ps_b")
                    first_mm = None
                    for et in range(nE):
                        for h in range(2):
                            hs = slice(h * 512, (h + 1) * 512)
                            mm = nc.tensor.matmul(en_ps[:, hs], HpT[:, et, ts],
                                                  encT_sb[:, et, hs],
                                                  start=(et == 0),
                                                  stop=(et == nE - 1))
                            if first_mm is None:
                                first_mm = mm
                            last_mm = mm
                    # keep mm2 groups sequential on PE: otherwise the
                    # scheduler interleaves groups and delays group 0's stop
                    # (and with it every softmax chain) by ~8us.
                    if prev_last_mm is not None:
                        _add_dep(first_mm.ins, prev_last_mm.ins, sync=False,
                                 reason="mm2 group order")
                    prev_last_mm = last_mm

                    x = xs.tile([P, SRC], F32, tag="x")
                    nc.vector.tensor_mul(x[:], en_ps[:], maskb[:])
                    ae_bf = sm.tile([P, SRC], BF16, tag="ae_bf")
                    nc.scalar.copy(ae_bf[:], x[:])
                    nc.sync.dma_start(out=ae_d[b, ts, :], in_=ae_bf[:])
                    negm = sm.tile([P, 1], F32, tag="negm")
                    nc.vector.tensor_reduce(negm[:], x[:], axis=AXL.X,
                                            op=ALU.max, negate=True)
                    ex = sm.tile([P, SRC], F32, tag="ex")
                    nc.scalar.activation(ex[:], x[:], ACT_EXP, bias=negm[:],
                                         scale=1.0)
                    rowsum = sm.tile([P, 1], F32, tag="rowsum")
                    nc.vector.scalar_tensor_tensor(ex[:], ex[:], 1.0, maskb[:],
                                                   op0=ALU.mult, op1=ALU.mult,
                                                   accum_out=rowsum[:])
                    z = sm.tile([P, 1], F32, tag="z")
                    nc.vector.tensor_scalar_add(z[:], rowsum[:], 1e-6)
                    rz = sm.tile([P, 1], F32, tag="rz")
                    nc.vector.reciprocal(rz[:], z[:])
                    attn = xs.tile([P, SRC], BF16, tag="attn")
                    nc.vector.tensor_scalar_mul(attn[:], ex[:], rz[:])
                    nc.sync.dma_start(out=aw_d[b, ts, :], in_=attn[:])
                    attns.append(attn)
                return attns

            def emit_tail(b, attns, val_sb, filler_chunks):
                """Per t-tile: PE transposes of attn + mm3; interleave the
                next batch's mm1 chunks between t-tiles."""
                for tt in range(nT):
                    ts = slice(tt * P, (tt + 1) * P)
                    attn = attns[tt]
                    attnT = sm.tile([P, nS, P], BF16, tag="attnT")
                    for st in range(nS):
                        pt = psA.tile([P, TRG], F32, tag="ps_a")
                        ptb = pt[:].bitcast(BF16)
                        nc.tensor.transpose(ptb[:, :P],
                                            attn[:, st * P:(st + 1) * P],
                                            identb[:])
                        if st % 2 == 0:
                            nc.vector.tensor_copy(attnT[:, st, :], ptb[:, :P])
                        else:
                            nc.scalar.copy(attnT[:, st, :], ptb[:, :P])

                    ctx_ps = psB.tile([P, TRGD], F32, tag="ps_b")
                    for st in range(nS):
                        for h in range(2):
                            hs = slice(h * 512, (h + 1) * 512)
                            nc.tensor.matmul(ctx_ps[:, hs], attnT[:, st, :],
                                             val_sb[:, st, hs],
                                             start=(st == 0),
                                             stop=(st == nS - 1))
                    ctx_sb = sm.tile([P, TRGD], BF16, tag="ctx_sb")
                    nc.scalar.copy(ctx_sb[:], ctx_ps[:])
                    nc.sync.dma_start(out=ctx_d[b, ts, :], in_=ctx_sb[:])

                    if filler_chunks:
                        filler_chunks.pop(0)()
                for ch in filler_chunks:
                    ch()

            # ---- two-batch pipeline ----
            hidT0, maskb0, encT0, val0 = emit_loads(0)
            HpT0 = emit_mm1_ramp(hidT0)
            attns0 = emit_mm2_chains(0, HpT0, maskb0, encT0)

            hidT1, maskb1, encT1, val1 = emit_loads(1)
            HpT1, chunks1 = mm1_chunks(hidT1)
            emit_tail(0, attns0, val0, chunks1)

            attns1 = emit_mm2_chains(1, HpT1, maskb1, encT1)
            emit_tail(1, attns1, val1, [])

    nc.compile()
    return nc


def kernel(hidden, encoder_outputs, encoder_value, encoder_mask, W):
    global LAST_EXEC_NS, LAST_RESULTS
    from concourse.bass_utils import run_bass_kernel_spmd

    if "nc" not in _cache:
        _cache["nc"] = _build()
    nc = _cache["nc"]

    hidden = np.ascontiguousarray(hidden, dtype=np.float32)
    encoder_outputs = np.ascontiguousarray(encoder_outputs, dtype=np.float32)
    encoder_value = np.ascontiguousarray(encoder_value, dtype=np.float32)
    encoder_mask = np.ascontiguousarray(encoder_mask, dtype=np.float32)
    W = np.ascontiguousarray(W, dtype=np.float32)

    in_maps = []
    for c in range(NCORES):
        sl = slice(c * BPC, (c + 1) * BPC)
        in_maps.append({
            "hidT": np.ascontiguousarray(hidden[sl].transpose(0, 2, 1)),
            "w": W,
            "encT": np.ascontiguousarray(encoder_outputs[sl].transpose(0, 2, 1)),
            "val": encoder_value[sl].astype(ml_dtypes.bfloat16),
            "mask": encoder_mask[sl][:, None, :],
        })

    trace = bool(int(os.environ.get("KERNEL_TRACE", "0")))
    res = run_bass_kernel_spmd(nc, in_maps, core_ids=list(range(NCORES)),
                               trace=trace)
    LAST_EXEC_NS = res.exec_time_ns
    LAST_RESULTS = res

    context = np.concatenate([res.results[c]["ctx"] for c in range(NCORES)],
                             axis=0).astype(np.float32)
    attn_weights = np.concatenate([res.results[c]["aw"] for c in range(NCORES)],
                                  axis=0).astype(np.float32)
    attn_energies = np.concatenate([res.results[c]["ae"] for c in range(NCORES)],
                                   axis=0).astype(np.float32)
    return context, attn_weights, attn_energies
